# revision 1
# baseline (speedup 1.0000x reference)
"""GCGRU cell on 8 TRN2 cores — fp8 DoubleRow convs + host-transposed diffusion.

Data-parallel over batch (512/core, padded to 516 = 86 six-batch quanta).
Host pre-packs: z^T (fp8, [108, 86, 2, 192]) so no pass-1 transposes on device;
block-diag [A|A2|A3] DoubleRow rhs; two-level (hi+lo/16) fp8 weights and z so
the 768-K convs run as fp8 DoubleRow (0.5 cyc/row) within 2e-2 accuracy.
Per-gate pre-acts accumulate in PSUM; sigmoid/tanh on Act engine with folded
per-row scale+bias; elementwise on DVE/Pool in bf16; bf16 output upcast on host.
"""
import numpy as np
import ml_dtypes

import concourse.bacc as bacc
import concourse.mybir as mybir
from concourse.tile import TileContext
from concourse.bass_utils import run_bass_kernel_spmd

N_CORES = 8
B, DX, U, NN = 4096, 64, 128, 36
BS = B // N_CORES            # 512
BSP = 516                    # padded to 86 six-batch quanta
NQ = BSP // 6                # 86
F32, BF16, F8 = mybir.dt.float32, mybir.dt.bfloat16, mybir.dt.float8e4
E4 = ml_dtypes.float8_e4m3
BF = ml_dtypes.bfloat16

MACRO_QP = [8] * 10 + [6]    # 10 macros of 48 batches + 1 of 36
assert sum(MACRO_QP) == NQ

SIG = mybir.ActivationFunctionType.Sigmoid
TANH = mybir.ActivationFunctionType.Tanh
DR = mybir.MatmulPerfMode.DoubleRow


def _q8(a):
    return np.asarray(a, np.float32).astype(E4)


# ---------------- host packing ----------------

def _prep_consts(adj, W_f, b_f, W_u, b_u, W_c, b_c):
    A = np.asarray(adj, np.float64).T
    Ss, sks = [], []
    for k in (1, 2, 3):
        Ak = np.linalg.matrix_power(A, k)
        cn = np.linalg.norm(Ak, axis=0).mean()
        sk = 2.0 ** np.round(np.log2(3.0 / cn))
        Ss.append(_q8(Ak * sk).astype(np.float32))
        sks.append(sk)
    # mbdDR [108, 2, 648]: rhs[36*bh+n, i, (g,k,bh,w)] = S_k[n,w]*(i==g, bh'==bh)
    mbd = np.zeros((108, 2, 2, 3, 3, 36), np.float32)
    for g in range(2):
        for k in range(3):
            for bh in range(3):
                mbd[bh * 36:(bh + 1) * 36, g, g, k, bh, :] = Ss[k]
    mbd = mbd.reshape(108, 2, 648)

    def pack(W):
        Wb = np.asarray(W, np.float64).reshape(128, 4, 192).copy()
        for k in range(3):
            Wb[:, k + 1, :] /= sks[k]
        Wf_ = Wb.reshape(128, 768)
        so = 120.0 / np.abs(Wf_).max(axis=1)
        Wr = Wf_ * so[:, None]
        Whi = _q8(Wr).astype(np.float32)
        Wlo = _q8((Wr - Whi) * 16.0).astype(np.float32) / 16.0  # stored /16
        return Whi.reshape(128, 4, 192), Wlo.reshape(128, 4, 192), so

    Fhi, Flo, sof = pack(W_f)
    Uhi, Ulo, sou = pack(W_u)
    Chi, Clo, soc = pack(W_c)
    Z64 = np.zeros((128, 64), np.float32)

    def xg(Wq, k):
        return Wq[:, k, 0:64]

    def hg(Wq, k):
        return Wq[:, k, 64:192]

    def st(c0, c1):      # -> [2, 128 in, 128 out]
        return np.stack([np.ascontiguousarray(c0.T), np.ascontiguousarray(c1.T)])

    def cat(a, b):
        return np.concatenate([a, b], axis=1)

    # G1 slots: 0 h8, 1 [x|g1x], 2 [xlo|g2x], 3 [0|g3x], 4 g1h, 5 g2h, 6 g3h,
    #           7 hlo.   G2 slots: 0 rh8, 1 g1rh, 2 g2rh, 3 g3rh, 4 rhlo
    def fu_instrs(Whi, Wlo, with_wlo):
        L = [
            ((0, 1), 1, st(hg(Whi, 0), cat(xg(Whi, 0), xg(Whi, 1)))),
            ((2, 3), 1, st(cat(xg(Whi, 0) / 16.0, xg(Whi, 2)),
                           cat(Z64, xg(Whi, 3)))),
            ((4, 5), 1, st(hg(Whi, 1), hg(Whi, 2))),
            ((6, 7), 1, st(hg(Whi, 3), hg(Whi, 0) / 16.0)),
        ]
        if with_wlo:
            L.append(((0, 1), 1, st(hg(Wlo, 0), cat(xg(Wlo, 0), Z64))))
        return L

    c_list = [
        ((1, 2), 1, st(cat(xg(Chi, 0), xg(Chi, 1)),
                       cat(xg(Chi, 0) / 16.0, xg(Chi, 2)))),
        ((1, 3), 1, st(cat(xg(Clo, 0), Z64), cat(Z64, xg(Chi, 3)))),
        ((0, 1), 2, st(hg(Chi, 0), hg(Chi, 1))),
        ((2, 3), 2, st(hg(Chi, 2), hg(Chi, 3))),
        ((0, 4), 2, st(hg(Clo, 0), hg(Chi, 0) / 16.0)),
    ]
    instrs = fu_instrs(Fhi, Flo, False) + fu_instrs(Uhi, Ulo, True) + c_list
    wall = np.stack([w for _, _, w in instrs])               # [14, 2, in, out]
    wall = np.ascontiguousarray(wall.transpose(2, 0, 1, 3))  # [128in, 14, 2, 128]
    pairs = [(p, t) for p, t, _ in instrs]

    scl = np.stack([1.0 / sof, np.asarray(b_f, np.float32),
                    1.0 / sou, np.asarray(b_u, np.float32),
                    1.0 / soc, np.asarray(b_c, np.float32)], axis=1)
    return {
        "mbd": _q8(mbd), "wall": _q8(wall), "scl": scl.astype(np.float32),
        "ident": np.eye(128, dtype=BF),
    }, pairs


def _prep_core(x, h):
    xp = np.zeros((BSP, DX, NN), np.float32)
    hp = np.zeros((BSP, U, NN), np.float32)
    xp[:BS], hp[:BS] = x, h
    x8 = _q8(xp)
    xlo = _q8((xp - x8.astype(np.float32)) * 16.0)
    h8 = _q8(hp)
    hlo = _q8((hp - h8.astype(np.float32)) * 16.0)
    d = {
        "h16": np.ascontiguousarray(hp.transpose(1, 0, 2)).astype(BF),
        "h8": np.ascontiguousarray(h8.transpose(1, 0, 2)),
        "hlo": np.ascontiguousarray(hlo.transpose(1, 0, 2)),
        "x8": np.ascontiguousarray(x8.transpose(1, 0, 2)),
        "xlo": np.ascontiguousarray(xlo.transpose(1, 0, 2)),
    }
    z8 = np.concatenate([x8, h8], axis=1).astype(np.float32)  # [516, 192, 36]
    zt = z8.reshape(NQ, 2, 3, 192, NN).transpose(2, 4, 0, 1, 3)  # bh n q i c
    d["zt"] = _q8(np.ascontiguousarray(zt.reshape(3, NN, NQ, 2, 192)
                                       .reshape(108, NQ, 2, 192)))
    return d


# ---------------- device build ----------------

def _build(pairs):
    nc = bacc.Bacc("TRN2", target_bir_lowering=False, debug=False,
                   num_devices=N_CORES)
    dp = nc.declare_dram_parameter
    d_h16 = dp("h16", [U, BSP, NN], BF16, isOutput=False)
    d_h8 = dp("h8", [U, BSP, NN], F8, isOutput=False)
    d_hlo = dp("hlo", [U, BSP, NN], F8, isOutput=False)
    d_x8 = dp("x8", [DX, BSP, NN], F8, isOutput=False)
    d_xlo = dp("xlo", [DX, BSP, NN], F8, isOutput=False)
    d_zt = dp("zt", [108, NQ, 2, 192], F8, isOutput=False)
    d_mbd = dp("mbd", [108, 2, 648], F8, isOutput=False)
    d_wall = dp("wall", [128, 14, 2, 128], F8, isOutput=False)
    d_scl = dp("scl", [128, 6], F32, isOutput=False)
    d_id = dp("ident", [128, 128], BF16, isOutput=False)
    d_out = dp("out", [U, BSP, NN], BF16, isOutput=True)

    with TileContext(nc) as tc:
        with (
            tc.tile_pool(name="consts", bufs=1) as cpool,
            tc.tile_pool(name="macro", bufs=3) as mpool,
            tc.tile_pool(name="wave", bufs=2, space="PSUM") as ps_wave,
            tc.tile_pool(name="pconv", bufs=2, space="PSUM") as ps_conv,
        ):
            mbd = cpool.tile([108, 2, 648], F8, name="mbd")
            wall = cpool.tile([128, 14, 2, 128], F8, name="wall")
            scl = cpool.tile([128, 6], F32, name="scl")
            ident = cpool.tile([128, 128], BF16, name="ident")
            for dst, src in ((mbd, d_mbd), (wall, d_wall), (scl, d_scl),
                             (ident, d_id)):
                nc.sync.dma_start(out=dst[:], in_=src[:])

            b0 = 0
            mac = []
            for mi, qpn in enumerate(MACRO_QP):
                MB = qpn * 6
                mac.append((mi, qpn, MB, MB * NN, b0 // 6,
                            slice(b0, b0 + MB)))
                b0 += MB

            def stage_a(m):
                mi, qpn, MB, T, q0, bsl = m
                G1 = mpool.tile([128, 8, T], F8, tag="G1", name=f"G1_{mi}")
                G2 = mpool.tile([128, 5, T], F8, tag="G2", name=f"G2_{mi}")
                h16 = mpool.tile([128, T], BF16, tag="h16", name=f"h16_{mi}")
                zt = mpool.tile([108, qpn, 2, 192], F8, tag="zt",
                                name=f"zt_{mi}")
                nc.sync.dma_start(out=zt[:], in_=d_zt[:, q0:q0 + qpn])
                for qp in range(qpn):
                    ph = ps_wave.tile([128, 2, 512], F32, tag="wave",
                                      name=f"ph_{mi}_{qp}")
                    lh = zt[:, qp, :, 64:192]
                    nc.tensor.matmul(ph[:, 0, 0:324], lh, mbd[:, :, 0:324],
                                     perf_mode=DR)
                    nc.tensor.matmul(ph[:, 1, 0:324], lh, mbd[:, :, 324:648],
                                     perf_mode=DR)
                    src = ph[:, :, 0:324].rearrange("p g (k v) -> p k g v",
                                                    k=3)
                    dst = G1[:, 4:7, qp * 216:(qp + 1) * 216].rearrange(
                        "p s (g v) -> p s g v", g=2)
                    nc.vector.tensor_copy(dst, src)
                for qp in range(qpn):
                    px = ps_wave.tile([64, 2, 512], F32, tag="wave",
                                      name=f"px_{mi}_{qp}")
                    lx = zt[:, qp, :, 0:64]
                    nc.tensor.matmul(px[:, 0, 0:324], lx,
                                     mbd[:, :, 0:324], perf_mode=DR)
                    nc.tensor.matmul(px[:, 1, 0:324], lx,
                                     mbd[:, :, 324:648], perf_mode=DR)
                    src = px[:, :, 0:324].rearrange("p g (k v) -> p k g v",
                                                    k=3)
                    dst = G1[64:128, 1:4, qp * 216:(qp + 1) * 216].rearrange(
                        "p s (g v) -> p s g v", g=2)
                    nc.scalar.copy(dst, src)
                nc.sync.dma_start(
                    out=h16[:].rearrange("c (b n) -> c b n", b=MB),
                    in_=d_h16[:, bsl])
                nc.sync.dma_start(out=G1[:, 0, :].rearrange(
                    "c (b n) -> c b n", b=MB), in_=d_h8[:, bsl])
                nc.sync.dma_start(out=G1[:, 7, :].rearrange(
                    "c (b n) -> c b n", b=MB), in_=d_hlo[:, bsl])
                nc.sync.dma_start(out=G1[0:64, 1, :].rearrange(
                    "c (b n) -> c b n", b=MB), in_=d_x8[:, bsl])
                nc.sync.dma_start(out=G1[0:64, 2, :].rearrange(
                    "c (b n) -> c b n", b=MB), in_=d_xlo[:, bsl])
                nc.vector.memzero(G1[0:64, 3, :])
                return m, G1, G2, h16, zt

            def stage_b1(st):
                (mi, qpn, MB, T, q0, bsl), G1, G2, h16, zt = st
                rr = mpool.tile([128, T], BF16, tag="rr", name=f"rr_{mi}")
                uu = mpool.tile([128, T], BF16, tag="uu", name=f"uu_{mi}")
                rh16 = mpool.tile([128, T], BF16, tag="rh16", name=f"rh_{mi}")
                t16 = mpool.tile([128, T], BF16, tag="t16", name=f"t16_{mi}")
                rhT = mpool.tile([108, qpn, 2, 128], F8, tag="rhT",
                                 name=f"rhT_{mi}")
                NT = T // 432

                def conv(lo, hi, dst16, func, si, tmin=0, tmax=None):
                    t = tmin
                    tend = NT if tmax is None else tmax
                    while t < tend:
                        tt = min(2, tend - t)
                        pc = ps_conv.tile([128, 2, 512], F32, tag="pc",
                                          name=f"pc_{mi}_{si}_{t}")
                        for j in range(tt):
                            cols = slice((t + j) * 432, (t + j + 1) * 432)
                            n = hi - lo
                            for i, (pr, gt) in enumerate(pairs[lo:hi]):
                                Gt = G1 if gt == 1 else G2
                                step = pr[1] - pr[0]
                                rhs = Gt[:, pr[0]:pr[1] + 1:step, cols]
                                nc.tensor.matmul(
                                    pc[:, j, 0:432], wall[:, lo + i], rhs,
                                    perf_mode=DR,
                                    start=(i == 0), stop=(i == n - 1))
                        if tt == 2:
                            src = pc[:, :, 0:432]
                            dstv = dst16[:, t * 432:(t + 2) * 432].rearrange(
                                "p (s x) -> p s x", s=2)
                        else:
                            src = pc[:, 0, 0:432]
                            dstv = dst16[:, t * 432:(t + 1) * 432]
                        nc.scalar.activation(dstv, src, func,
                                             bias=scl[:, si + 1:si + 2],
                                             scale=scl[:, si:si + 1])
                        t += tt

                conv(0, 4, rr, SIG, 0)
                nc.vector.tensor_mul(rh16[:], rr[:], h16[:])
                nc.scalar.copy(G2[:, 0, :], rh16[:])

                def rest():
                    _rest(st, uu, rh16, t16, rhT, conv)
                return st, uu, conv, rest

            def _rest(st, uu, rh16, t16, rhT, conv):
                (mi, qpn, MB, T, q0, bsl), G1, G2, h16, zt = st
                NT = T // 432

                def pass2_qp(qp):
                    ptr = ps_conv.tile([108, 2, 128], BF16, tag="pc",
                                       name=f"ptr_{mi}_{qp}")
                    for j in range(2):
                        g3 = slice((qp * 2 + j) * 108, (qp * 2 + j + 1) * 108)
                        nc.tensor.transpose(ptr[:, j, :], rh16[:, g3],
                                            ident[:])
                    if qp % 2 == 0:
                        nc.scalar.copy(rhT[:, qp], ptr[:])
                    else:
                        nc.vector.tensor_copy(rhT[:, qp], ptr[:])
                    prh = ps_wave.tile([128, 2, 512], F32, tag="wave",
                                       name=f"prh_{mi}_{qp}")
                    nc.tensor.matmul(prh[:, 0, 0:324], rhT[:, qp],
                                     mbd[:, :, 0:324], perf_mode=DR)
                    nc.tensor.matmul(prh[:, 1, 0:324], rhT[:, qp],
                                     mbd[:, :, 324:648], perf_mode=DR)
                    src = prh[:, :, 0:324].rearrange("p g (k v) -> p k g v",
                                                     k=3)
                    dst = G2[:, 1:4, qp * 216:(qp + 1) * 216].rearrange(
                        "p s (g v) -> p s g v", g=2)
                    if qp % 2 == 0:
                        nc.vector.tensor_copy(dst, src)
                    else:
                        nc.scalar.copy(dst, src)

                half = (qpn + 1) // 2
                conv(4, 9, uu, SIG, 2, tmax=NT // 2)
                for qp in range(half):
                    pass2_qp(qp)
                conv(4, 9, uu, SIG, 2, tmin=NT // 2)
                for qp in range(half, qpn):
                    pass2_qp(qp)
                nc.vector.tensor_sub(t16[:], rh16[:], G2[:, 0, :])
                nc.vector.tensor_scalar_mul(G2[:, 4, :], t16[:], 16.0)

            def stage_b2(bst):
                (m, G1, G2, h16, zt), uu, conv, _ = bst
                mi, qpn, MB, T, q0, bsl = m
                ct = mpool.tile([128, T], BF16, tag="ct", name=f"ct_{mi}")
                t2 = mpool.tile([128, T], BF16, tag="t2", name=f"t2_{mi}")
                oo = mpool.tile([128, T], BF16, tag="oo", name=f"oo_{mi}")
                conv(9, 14, ct, TANH, 4)
                nc.vector.tensor_sub(t2[:], h16[:], ct[:])
                nc.vector.tensor_mul(t2[:], t2[:], uu[:])
                nc.vector.tensor_add(oo[:], t2[:], ct[:])
                nc.sync.dma_start(
                    out=d_out[:, bsl],
                    in_=oo[:].rearrange("c (b n) -> c b n", b=MB))

            pa = None
            pb = None
            for m in mac:
                cur = stage_a(m)
                if pa is not None:
                    b = stage_b1(pa)
                    if pb is not None:
                        stage_b2(pb)
                    b[3]()
                    pb = b
                pa = cur
            b = stage_b1(pa)
            if pb is not None:
                stage_b2(pb)
            b[3]()
            stage_b2(b)
    nc.compile()
    return nc


_CACHE = {}
LAST_RESULTS = None


def kernel(x, h, adj, W_f, b_f, W_u, b_u, W_c, b_c):
    global LAST_RESULTS
    x = np.ascontiguousarray(x, np.float32)
    h = np.ascontiguousarray(h, np.float32)
    consts, pairs = _prep_consts(adj, W_f, b_f, W_u, b_u, W_c, b_c)
    if "nc" not in _CACHE:
        _CACHE["nc"] = _build(pairs)
    nc = _CACHE["nc"]
    in_maps = []
    for i in range(N_CORES):
        d = _prep_core(x[i * BS:(i + 1) * BS], h[i * BS:(i + 1) * BS])
        d.update(consts)
        in_maps.append(d)
    res = run_bass_kernel_spmd(nc, in_maps, list(range(N_CORES)))
    LAST_RESULTS = res
    outs = []
    for i in range(N_CORES):
        o = res.results[i]["out"]  # [128, 516, 36] bf16
        outs.append(np.asarray(o[:, :BS], np.float32).transpose(1, 0, 2))
    return np.concatenate(outs, axis=0)

